# revision 3
# baseline (speedup 1.0000x reference)
"""Trainium2 Bass kernel: batched int8 GEMM (bmm_s8t_s8n) with fused bf16 dequant.

Computes out[i] = bf16(alpha * (a[i] @ b[i]^T)) for a [32,512,2048] int8,
b [32,512,2048] int8 (both row-major with K innermost), alpha scalar fp32.

Strategy (per 8-core SPMD shard = 4 batches/core):
  1. Both operands need K on partitions for the PE.  DMA xbar transpose only
     supports 2-byte elements, so int8 pairs along K are viewed as uint16 and
     transposed chunk-by-chunk ([512 rows, 128 pair-cols] -> [128, 512]u16)
     into SBUF staging tiles.  A partition then holds two int8 k-slices
     byte-interleaved along the free dim; whatever (partition, col) mapping the
     xbar uses is identical for a and b, so the contraction is correct
     regardless.
  2. int8 -> bf16 de-interleave copies (DVE for a, ACT for b) produce
     [128, 1024] bf16 chunk tiles (two k-tiles each).  int8 is exact in bf16;
     fp32 PSUM accumulation matches int32 results.
  3. Fine-grained pipeline: per-chunk transposes and casts (64 of each) keep
     the PE fed at ~1.8us granularity; the xbar stream (~51us for 8.4MB at
     ~150GB/s) runs just ahead of the PE (~55us of matmuls at the 216ns
     N=512 issue floor).
  4. PE prewarm: ~8 dummy matmuls on a zeroed scratch tile run during the
     fixed ~7us NEFF preamble so the HAM clock gate reaches 2.4GHz before the
     first real matmul.
  5. t-major accumulation across 4 open PSUM banks per batch; each m-group's
     dequant+store fires immediately after its last k-tile matmul so the
     epilogues hide inside the matmul stream.
"""

from dataclasses import dataclass, replace

import numpy as np

import concourse.mybir as mybir
from concourse import bacc
from concourse.bass_utils import run_bass_kernel_spmd
from concourse.tile import TileContext

B, M, N, K = 32, 512, 512, 2048
NCORES = 8
BPC = B // NCORES  # batches per core
KP = K // 2  # uint16 pair-columns per row
PART = 128
NCHUNK = KP // PART  # transposed chunks per operand-batch (8)
KTILES = 2 * NCHUNK  # k-tiles of 128 per batch (16)


@dataclass(frozen=True)
class Cfg:
    stage_bufs: int = 3  # per tag [128, 4096]u16 staging tiles (8 KiB/partition)
    conv_bufs: int = 16  # per tag [128, 1024]bf16 chunk tiles (2 KiB/partition)
    prewarm: int = 8  # dummy N=512 matmuls to pull HAM to 2.4GHz early
    obuf_bufs: int = 8
    psum_bufs: int = 8
    deq_alt: bool = True  # alternate dequant between DVE and ACT by m-group
    tail_all: bool = True  # per-m epilogue right after its last mm (all batches)
    store_eng: str = "gpsimd"
    group_chunks: int = 1  # chunks per transpose call (1 = fully fine-grained)
    cast_chunks: int = 1  # chunks per cast instruction


VARIANTS = {
    "w1": Cfg(),
    "w2": Cfg(prewarm=0),
    "w3": Cfg(tail_all=False),
    "w4": Cfg(group_chunks=2, cast_chunks=2, conv_bufs=8),
    "w5": Cfg(prewarm=12),
    "w6": Cfg(prewarm=5),
}

_cfg = VARIANTS["w1"]


def set_variant(name):
    global _cfg
    _cfg = VARIANTS[name] if isinstance(name, str) else name


def _build(alpha: float, bpc: int = BPC):
    cfg = _cfg
    nc = bacc.Bacc("TRN2", target_bir_lowering=False)
    a_d = nc.dram_tensor("a", [bpc, M, KP], mybir.dt.uint16, kind="ExternalInput")
    b_d = nc.dram_tensor("b", [bpc, N, KP], mybir.dt.uint16, kind="ExternalInput")
    o_d = nc.dram_tensor("out", [bpc, M, N], mybir.dt.bfloat16, kind="ExternalOutput")

    gc = cfg.group_chunks
    cc = cfg.cast_chunks
    with TileContext(nc) as tc:
        with (
            tc.tile_pool(name="warm", bufs=1) as warm,
            tc.tile_pool(name="stage", bufs=cfg.stage_bufs) as stage,
            tc.tile_pool(name="conv", bufs=cfg.conv_bufs) as conv,
            tc.tile_pool(name="obuf", bufs=cfg.obuf_bufs) as obuf,
            tc.tile_pool(name="psum", bufs=cfg.psum_bufs, space="PSUM") as psum_pool,
        ):
            store_ring = getattr(nc, cfg.store_eng)
            # --- PE prewarm: no input deps, runs during the NEFF preamble ---
            if cfg.prewarm:
                wt = warm.tile([PART, N], mybir.dt.bfloat16)
                nc.vector.memset(wt[:, :], 0.0)
                wps = psum_pool.tile([PART, N], mybir.dt.float32, tag="ps")
                for _ in range(cfg.prewarm):
                    nc.tensor.matmul(
                        wps[:, :], wt[:, :PART], wt[:, :], start=True, stop=True
                    )

            for bi in range(bpc):
                sts = {}
                stt = {}
                for name in ("a", "b"):
                    st = stage.tile([PART, NCHUNK * M], mybir.dt.uint16, tag=f"st_{name}")
                    stt[name] = st
                    sts[name] = st[:, :].bitcast(mybir.dt.int8)  # [128, 2*KP]
                # per-chunk-group transposes, a/b interleaved so the PE's first
                # k-tiles (which need both operands) arrive soonest
                for c0 in range(0, NCHUNK, gc):
                    for name, dram in (("a", a_d), ("b", b_d)):
                        nc.sync.dma_start_transpose(
                            stt[name][:, c0 * M : (c0 + gc) * M].rearrange(
                                "q (c m) -> q c m", m=M
                            ),
                            dram[bi, :, c0 * PART : (c0 + gc) * PART],
                        )
                # per-chunk-group int8 -> bf16 de-interleave casts
                ktiles = {"a": [], "b": []}
                for c0 in range(0, NCHUNK, cc):
                    for name in ("a", "b"):
                        chunk8 = sts[name][:, c0 * 2 * M : (c0 + cc) * 2 * M]
                        eng = nc.vector if name == "a" else nc.scalar
                        bt = conv.tile(
                            [PART, cc * 2 * M], mybir.dt.bfloat16, tag=f"bf_{name}"
                        )
                        # in: [q][c][m][p] bytes -> iterate (c, p, m); out [c][p][m]
                        in_ap = chunk8.rearrange("q (c m p) -> q c p m", p=2, m=M)
                        out_ap = bt[:, :].rearrange("q (c p m) -> q c p m", m=M, p=2)
                        if eng is nc.scalar:
                            eng.copy(out=out_ap, in_=in_ap)
                        else:
                            eng.tensor_copy(out=out_ap, in_=in_ap)
                        for j in range(cc):
                            ktiles[name].append(bt[:, j * 2 * M : (j + 1) * 2 * M])
                n_mt = M // PART

                def mm(ps, mi, c, p, t):
                    nc.tensor.matmul(
                        ps[:, :],
                        ktiles["a"][c][:, p * M + mi * PART : p * M + (mi + 1) * PART],
                        ktiles["b"][c][:, p * N : (p + 1) * N],
                        start=(t == 0),
                        stop=(t == KTILES - 1),
                    )

                def epilogue(ps, mi):
                    ot = obuf.tile([PART, N], mybir.dt.bfloat16)
                    if cfg.deq_alt and mi % 2 == 1:
                        nc.scalar.activation(
                            ot[:, :],
                            ps[:, :],
                            mybir.ActivationFunctionType.Copy,
                            scale=float(alpha),
                        )
                    else:
                        nc.vector.tensor_scalar_mul(ot[:, :], ps[:, :], float(alpha))
                    store_ring.dma_start(o_d[bi, mi * PART : (mi + 1) * PART, :], ot[:, :])

                pss = [
                    psum_pool.tile([PART, N], mybir.dt.float32, name=f"ps_{bi}_{mi}", tag="ps")
                    for mi in range(n_mt)
                ]
                for t in range(KTILES - 1):
                    for mi in range(n_mt):
                        mm(pss[mi], mi, t // 2, t % 2, t)
                t = KTILES - 1
                if cfg.tail_all or bi == bpc - 1:
                    for mi in range(n_mt):
                        mm(pss[mi], mi, t // 2, t % 2, t)
                        epilogue(pss[mi], mi)
                else:
                    for mi in range(n_mt):
                        mm(pss[mi], mi, t // 2, t % 2, t)
                    for mi in range(n_mt):
                        epilogue(pss[mi], mi)
    nc.compile()
    return nc


def run(a, b, alpha, trace=False, repeats=1):
    """Run on 8 NeuronCores; returns (out [32,512,512] bf16, list[BassKernelResults])."""
    a = np.ascontiguousarray(np.asarray(a))
    b = np.ascontiguousarray(np.asarray(b))
    if a.dtype != np.int8:
        a = a.astype(np.int8)
    if b.dtype != np.int8:
        b = b.astype(np.int8)
    nc = _build(float(alpha))
    in_maps = []
    for ci in range(NCORES):
        sl = slice(ci * BPC, (ci + 1) * BPC)
        in_maps.append({"a": a[sl].view(np.uint16), "b": b[sl].view(np.uint16)})
    all_res = []
    for _ in range(repeats):
        res = run_bass_kernel_spmd(
            nc, in_maps, core_ids=list(range(NCORES)), trace=trace
        )
        all_res.append(res)
    out = np.concatenate([r["out"] for r in all_res[-1].results], axis=0)
    return out, all_res


def kernel(a, b, alpha):
    out, _ = run(a, b, alpha)
    return out


# revision 8
# speedup vs baseline: 1.4696x; 1.4696x over previous
"""Trainium2 Bass kernel: batched int8 GEMM (bmm_s8t_s8n) with fused bf16 dequant.

Computes out[i] = bf16(alpha * (a[i] @ b[i]^T)) for a [32,512,2048] int8,
b [32,512,2048] int8 (both row-major with K innermost), alpha scalar fp32.

Strategy (per 8-core SPMD shard = 4 batches/core):
  1. Both operands need K on partitions for the PE.  DMA xbar transpose only
     supports 2-byte elements, so int8 pairs along K are viewed as uint16 and
     transposed chunk-by-chunk ([512 rows, 128 pair-cols] -> [128, 512]u16)
     into SBUF staging tiles.  A partition then holds two int8 k-slices
     byte-interleaved along the free dim; whatever (partition, col) mapping the
     xbar uses is identical for a and b, so the contraction is correct
     regardless.
  2. The xbar stream (~51us for 8.4MB at ~160GB/s) is the feed floor and the
     PE (~55us of matmuls at the 216ns N=512 issue floor) the compute floor;
     they overlap almost entirely.  Descriptor generation costs ~0.9us/call
     fixed + ~0.33us/chunk and must come from a single engine (concurrent
     transpose issue from two HWDGE engines races the xbar's shared base
     register and corrupts data - observed).  All gens go on SYNC: 2-chunk
     calls (~0.8us/chunk gen = xbar rate) keep the stream gapless; batch 0's
     first two chunks go per-chunk so the first matmul issues at ~11us.
  3. int8 -> bf16 de-interleave casts run at matching granularity on DVE
     (~0.7us/chunk) so k-tiles trail the stream by under a microsecond.
  4. PE prewarm: dummy matmuls on a zeroed scratch tile run during the fixed
     ~7us NEFF preamble so the HAM clock gate reaches 2.4GHz around when the
     first real matmul issues (~11us).
  5. t-major accumulation across 4 open PSUM banks per batch; dequant+store
     epilogues are emitted AFTER the next batch's casts (flush-late) so they
     never head-of-line-block the cast/gen queues; the final m-group's
     epilogue is split in half across DVE+ACT to shorten the tail.
"""

from dataclasses import dataclass

import numpy as np

import concourse.mybir as mybir
from concourse import bacc
from concourse.bass_utils import run_bass_kernel_spmd
from concourse.tile import TileContext

B, M, N, K = 32, 512, 512, 2048
NCORES = 8
BPC = B // NCORES  # batches per core
KP = K // 2  # uint16 pair-columns per row
PART = 128
NCHUNK = KP // PART  # transposed chunks per operand-batch (8)
KTILES = 2 * NCHUNK  # k-tiles of 128 per batch (16)


SCHED0 = (1, 1, 2, 2, 2)  # batch-0 chunk-group sizes (gens and casts)
SCHED = (2, 2, 2, 2)  # later batches


@dataclass(frozen=True)
class Cfg:
    stage_bufs: int = 3  # per tag [128, 4096]u16 staging tiles (8 KiB/partition)
    conv_bufs: int = 10  # per tag bf16 chunk tiles (up to 4 KiB/partition)
    prewarm: int = 8  # dummy N=512 matmuls to pull HAM to 2.4GHz early
    obuf_bufs: int = 8
    psum_bufs: int = 8
    split_tail: bool = True  # split final m-group epilogue in half across DVE+ACT
    b_casts: str = "vector"  # engine for b-operand casts ("vector"|"scalar"|"alt")
    sched0: tuple = SCHED0
    sched: tuple = SCHED


VARIANTS = {
    "w1": Cfg(),
    "w2": Cfg(prewarm=0),
    "w3": Cfg(b_casts="alt"),
    "w4": Cfg(sched0=(4, 4), sched=(4, 4), conv_bufs=6),
    "w5": Cfg(prewarm=12),
    "w6": Cfg(split_tail=False),
    "w7": Cfg(sched0=(1, 1, 1, 1, 2, 2), sched=(2, 2, 2, 2)),
    "w8": Cfg(sched0=(1, 1, 2, 4), sched=(2, 2, 2, 2)),
}

_cfg = VARIANTS["w1"]


def set_variant(name):
    global _cfg
    _cfg = VARIANTS[name] if isinstance(name, str) else name


def _build(alpha: float, bpc: int = BPC):
    cfg = _cfg
    nc = bacc.Bacc("TRN2", target_bir_lowering=False)
    a_d = nc.dram_tensor("a", [bpc, M, KP], mybir.dt.uint16, kind="ExternalInput")
    b_d = nc.dram_tensor("b", [bpc, N, KP], mybir.dt.uint16, kind="ExternalInput")
    o_d = nc.dram_tensor("out", [bpc, M, N], mybir.dt.bfloat16, kind="ExternalOutput")

    n_mt = M // PART
    with TileContext(nc) as tc:
        with (
            tc.tile_pool(name="warm", bufs=1) as warm,
            tc.tile_pool(name="stage", bufs=cfg.stage_bufs) as stage,
            tc.tile_pool(name="conv", bufs=cfg.conv_bufs) as conv,
            tc.tile_pool(name="obuf", bufs=cfg.obuf_bufs) as obuf,
            tc.tile_pool(name="psum", bufs=cfg.psum_bufs, space="PSUM") as psum_pool,
        ):
            # --- PE prewarm: no input deps, runs during the NEFF preamble ---
            if cfg.prewarm:
                wt = warm.tile([PART, N], mybir.dt.bfloat16)
                nc.vector.memset(wt[:, :], 0.0)
                wps = psum_pool.tile([PART, N], mybir.dt.float32, tag="ps")
                for _ in range(cfg.prewarm):
                    nc.tensor.matmul(
                        wps[:, :], wt[:, :PART], wt[:, :], start=True, stop=True
                    )

            pending = []  # flush-late epilogues: (psum, bi, mi, deq_eng)

            def epilogue(ps, bi, mi, deq_eng, lo=0, hi=N):
                ot = obuf.tile([PART, N], mybir.dt.bfloat16)
                if deq_eng is nc.scalar:
                    nc.scalar.activation(
                        ot[:, lo:hi],
                        ps[:, lo:hi],
                        mybir.ActivationFunctionType.Copy,
                        scale=float(alpha),
                    )
                else:
                    deq_eng.tensor_scalar_mul(ot[:, lo:hi], ps[:, lo:hi], float(alpha))
                nc.gpsimd.dma_start(
                    o_d[bi, mi * PART : (mi + 1) * PART, lo:hi], ot[:, lo:hi]
                )

            for bi in range(bpc):
                sched = cfg.sched0 if bi == 0 else cfg.sched
                assert sum(sched) == NCHUNK
                sts = {}
                stt = {}
                for name in ("a", "b"):
                    st = stage.tile([PART, NCHUNK * M], mybir.dt.uint16, tag=f"st_{name}")
                    stt[name] = st
                    sts[name] = st[:, :].bitcast(mybir.dt.int8)  # [128, 2*KP]
                # transposes: all on SYNC (single issuer - the xbar path has
                # shared state), a/b interleaved per chunk-group
                c0 = 0
                for g in sched:
                    for name, dram in (("a", a_d), ("b", b_d)):
                        nc.sync.dma_start_transpose(
                            stt[name][:, c0 * M : (c0 + g) * M].rearrange(
                                "q (c m) -> q c m", m=M
                            ),
                            dram[bi, :, c0 * PART : (c0 + g) * PART],
                        )
                    c0 += g
                # casts: matching granularity on DVE (arrival order a,b)
                ktiles = {"a": [], "b": []}
                c0 = 0
                for g in sched:
                    for name in ("a", "b"):
                        chunk8 = sts[name][:, c0 * 2 * M : (c0 + g) * 2 * M]
                        if name == "a" or cfg.b_casts == "vector":
                            eng = nc.vector
                        elif cfg.b_casts == "alt":
                            eng = nc.vector if c0 % 2 == 0 else nc.scalar
                        else:
                            eng = nc.scalar
                        bt = conv.tile(
                            [PART, g * 2 * M], mybir.dt.bfloat16, tag=f"bf_{name}"
                        )
                        # in: [q][c][m][p] bytes -> iterate (c, p, m); out [c][p][m]
                        in_ap = chunk8.rearrange("q (c m p) -> q c p m", p=2, m=M)
                        out_ap = bt[:, :].rearrange("q (c p m) -> q c p m", m=M, p=2)
                        if eng is nc.scalar:
                            eng.copy(out=out_ap, in_=in_ap)
                        else:
                            eng.tensor_copy(out=out_ap, in_=in_ap)
                        for j in range(g):
                            ktiles[name].append(bt[:, j * 2 * M : (j + 1) * 2 * M])
                    c0 += g

                def mm(ps, mi, c, p, t):
                    nc.tensor.matmul(
                        ps[:, :],
                        ktiles["a"][c][:, p * M + mi * PART : p * M + (mi + 1) * PART],
                        ktiles["b"][c][:, p * N : (p + 1) * N],
                        start=(t == 0),
                        stop=(t == KTILES - 1),
                    )

                # flush previous batch's epilogues now that this batch's casts
                # are queued (they only wait on the previous batch's last mms)
                for ps, pbi, mi, eng in pending:
                    epilogue(ps, pbi, mi, eng)
                pending = []

                pss = [
                    psum_pool.tile(
                        [PART, N], mybir.dt.float32, name=f"ps_{bi}_{mi}", tag="ps"
                    )
                    for mi in range(n_mt)
                ]
                for t in range(KTILES - 1):
                    for mi in range(n_mt):
                        mm(pss[mi], mi, t // 2, t % 2, t)
                t = KTILES - 1
                if bi < bpc - 1:
                    for mi in range(n_mt):
                        mm(pss[mi], mi, t // 2, t % 2, t)
                        pending.append(
                            (pss[mi], bi, mi, nc.vector if mi % 2 == 0 else nc.scalar)
                        )
                else:
                    # final batch: epilogue immediately after each m-group's
                    # last matmul; split the very last one across DVE+ACT
                    for mi in range(n_mt):
                        mm(pss[mi], mi, t // 2, t % 2, t)
                        if mi < n_mt - 1 or not cfg.split_tail:
                            epilogue(
                                pss[mi], bi, mi,
                                nc.vector if mi % 2 == 0 else nc.scalar,
                            )
                        else:
                            epilogue(pss[mi], bi, mi, nc.vector, 0, N // 2)
                            epilogue(pss[mi], bi, mi, nc.scalar, N // 2, N)
    nc.compile()
    return nc


def run(a, b, alpha, trace=False, repeats=1):
    """Run on 8 NeuronCores; returns (out [32,512,512] bf16, list[BassKernelResults])."""
    a = np.ascontiguousarray(np.asarray(a))
    b = np.ascontiguousarray(np.asarray(b))
    if a.dtype != np.int8:
        a = a.astype(np.int8)
    if b.dtype != np.int8:
        b = b.astype(np.int8)
    nc = _build(float(alpha))
    in_maps = []
    for ci in range(NCORES):
        sl = slice(ci * BPC, (ci + 1) * BPC)
        in_maps.append({"a": a[sl].view(np.uint16), "b": b[sl].view(np.uint16)})
    all_res = []
    for _ in range(repeats):
        res = run_bass_kernel_spmd(
            nc, in_maps, core_ids=list(range(NCORES)), trace=trace
        )
        all_res.append(res)
    out = np.concatenate([r["out"] for r in all_res[-1].results], axis=0)
    return out, all_res


def kernel(a, b, alpha):
    out, _ = run(a, b, alpha)
    return out
